# revision 1
# baseline (speedup 1.0000x reference)
"""Multi-head attention Trainium2 kernel (8-core head-parallel).

Problem: B=2, S=2048, D=1024, H=16 heads x HD=64.
Sharding: tensor-parallel over heads. Each core owns 2 heads (J=128 columns
of Wq/Wk/Wv, 128 rows of Wo) and computes the full sequence for both batches.
Each core produces a partial output (its heads' contribution through Wo);
the host sums the 8 partials and adds bo.

Inputs are fed pre-transposed (xT = X^T) so every matmul contracts over the
partition axis with no on-chip transposition of X. Per-core compute:
  Q^T/K^T/V^T = W^T @ X^T         (lhsT=W tiles, rhs=xT tiles, N=512)
  S^T[k,q]    = K^T_tile.T @ Q^T  (row-tiled head pairs: head A on
                                   partitions 0-63, head B on 64-127,
                                   running concurrently in the PE array)
  P = exp(S^T / 8)                (ScalarE, scale folded into activation;
                                   max-subtraction skipped: scores are
                                   ~N(0,1), exp never overflows fp32)
  out^T[d,q]  = V_aug.T @ P       (V with a ones-column appended so PSUM
                                   row 64 accumulates the softmax
                                   denominator for free)
  normalize   : denominator reciprocal broadcast to all partitions via an
                all-ones selector matmul, then one multiply per head
  partial out = aoT.T @ Wo_rows   (contraction over this core's 128 head
                                   dims; host sums partials across cores)

Scheduling: Tile's per-engine instruction order follows emission order, so
cross-phase overlap is created at the source level: projections for batch
b+1 are emitted interleaved into batch b's attention group loop (filling
the PE idle time under each exp), attn@V lags the exp pipeline by one
group, and the output projection for a q-block is emitted right after that
q-block's normalization.

PSUM budget (8 banks): ps_s 4 + ps_av 2 + 2x 1-bank scratch.
"""

import numpy as np

import concourse.bass as bass
import concourse.bacc as bacc
import concourse.tile as tile
import concourse.mybir as mybir
from concourse.masks import make_identity

F32 = mybir.dt.float32
F32R = mybir.dt.float32r

P = 128


def build_nc(
    S=2048,          # sequence length per batch
    D=1024,          # model dim
    DOUT=1024,       # output dim (cols of Wo)
    B=2,             # batches
    QB=512,          # q-block (moving free dim)
    mm_r=True,       # use float32r for the heavy matmuls
    rec_mm_r=True,   # use float32r for the denominator-broadcast matmul
    KG=1,            # k-tiles per exp group (group occupies KG*2 psum banks)
    expS_bufs=4,
    xin_bufs=2,
    out_bufs=6,
    aux_bufs=2,
    loop_n=None,     # wrap the body in a For_i loop (benchmark amplification)
):
    J = P            # head-columns per core (2 heads x 64)
    HD = 64
    DT = D // P      # contraction tiles for projections
    NQB = S // QB
    KT = S // P      # key tiles
    ST = S // P      # seq tiles
    NG = KT // KG    # exp groups per q-block
    SCALE = 1.0 / float(np.sqrt(HD))
    assert S % QB == 0 and D % P == 0 and KT % KG == 0 and DOUT % 512 == 0

    MDT = F32R if mm_r else F32          # dtype of matmul operands
    RDT = F32R if rec_mm_r else F32       # dtype of denominator-bcast operands

    nc = bacc.Bacc(None, target_bir_lowering=False)

    xT_h = nc.dram_tensor("xt", [D, B * S], MDT, kind="ExternalInput")
    wq_h = nc.dram_tensor("wq", [D, J], MDT, kind="ExternalInput")
    wk_h = nc.dram_tensor("wk", [D, J], MDT, kind="ExternalInput")
    wv_h = nc.dram_tensor("wv", [D, J], MDT, kind="ExternalInput")
    bq_h = nc.dram_tensor("bq", [J], F32, kind="ExternalInput")
    bk_h = nc.dram_tensor("bk", [J], F32, kind="ExternalInput")
    bv_h = nc.dram_tensor("bv", [J], F32, kind="ExternalInput")
    wo_h = nc.dram_tensor("wo", [J, DOUT], MDT, kind="ExternalInput")
    out_h = nc.dram_tensor("out", [B * S, DOUT], F32, kind="ExternalOutput")

    with tile.TileContext(nc) as tc:
        with (
            tc.tile_pool(name="const", bufs=1) as const,
            tc.tile_pool(name="xin", bufs=xin_bufs) as xin,
            tc.tile_pool(name="proj", bufs=2) as proj,
            tc.tile_pool(name="vtp", bufs=1) as vtp,
            tc.tile_pool(name="expp", bufs=expS_bufs) as expp,
            tc.tile_pool(name="aux", bufs=aux_bufs) as aux,
            tc.tile_pool(name="outp", bufs=out_bufs) as outp,
            tc.tile_pool(name="psq", bufs=1, space="PSUM") as psq,
            tc.tile_pool(name="psp", bufs=2, space="PSUM") as psp,
        ):
            # ---- constants ----
            wq_sb = const.tile([P, DT, J], MDT)
            wk_sb = const.tile([P, DT, J], MDT)
            wv_sb = const.tile([P, DT, J], MDT)
            nc.sync.dma_start(
                wk_sb[:], wk_h.ap().rearrange("(do di) j -> di do j", di=P)
            )
            for w_sb, w_h in ((wq_sb, wq_h), (wv_sb, wv_h)):
                nc.gpsimd.dma_start(
                    w_sb[:], w_h.ap().rearrange("(do di) j -> di do j", di=P)
                )
            wo_sb = const.tile([P, DOUT], MDT)
            nc.gpsimd.dma_start(wo_sb[:], wo_h.ap())
            bq_sb = const.tile([P, 1], F32)
            bk_sb = const.tile([P, 1], F32)
            bv_sb = const.tile([P, 1], F32)
            for b_sb, b_h in ((bq_sb, bq_h), (bk_sb, bk_h), (bv_sb, bv_h)):
                nc.gpsimd.dma_start(b_sb[:], b_h.ap().unsqueeze(-1))
            ident_f = const.tile([P, P], F32)
            make_identity(nc, ident_f[:])
            ident = const.tile([P, P], MDT)
            nc.vector.tensor_copy(out=ident[:], in_=ident_f[:])
            ones_f = const.tile([P, 1], F32)
            nc.vector.memset(ones_f[:], 1.0)
            # all-ones selector for the denominator broadcast matmul
            sel = const.tile([64, P], RDT)
            nc.vector.tensor_copy(
                out=sel[:], in_=ones_f[0:64, :].to_broadcast((64, P))
            )
            # denominator staging: row 0 holds head-A sums, row 32 head-B
            sums = const.tile([64, 2, QB], RDT)
            zeros_f = const.tile([P, 1], F32)
            nc.vector.memset(zeros_f[:], 0.0)
            nc.vector.tensor_copy(
                out=sums[:].rearrange("p a q -> p (a q)"),
                in_=zeros_f[0:64, :].to_broadcast((64, 2 * QB)),
            )

            # per-batch persistent tiles
            def alloc_batch_tiles():
                return {
                    "qT": proj.tile([P, S], MDT, tag="qT", name="qT"),
                    "kT": proj.tile([P, S], MDT, tag="kT", name="kT"),
                    "vT": vtp.tile([P, S], MDT, tag="vT", name="vT"),
                    "v_st": proj.tile([P, KT, 130], MDT, tag="v_st", name="v_st"),
                }

            def proj_units(bt, b):
                """Generator of emission units for batch b's projections.
                Unit = one projection chain or one per-sblk transpose group;
                each sblk's transposes follow its V chain so attention can
                begin after the first sblk."""
                v_st, vT = bt["v_st"], bt["vT"]
                ones_bc = ones_f[:].unsqueeze(1).to_broadcast((P, KT, 1))
                nc.vector.tensor_copy(out=v_st[:, :, 64:65], in_=ones_bc)
                nc.vector.tensor_copy(out=v_st[:, :, 129:130], in_=ones_bc)
                KPS = KT // NQB  # k-tiles produced per sblk
                for sblk in range(NQB):
                    xh = xin.tile([P, DT, QB], MDT, tag="xt_half")
                    xt_view = xT_h.ap().rearrange("(do di) s -> di do s", di=P)[
                        :, :, b * S + sblk * QB : b * S + (sblk + 1) * QB
                    ]
                    dma_eng = nc.sync if sblk % 2 == 0 else nc.gpsimd
                    for dh in range(2):
                        w = DT // 2
                        dma_eng.dma_start(
                            xh[:, dh * w : (dh + 1) * w, :],
                            xt_view[:, dh * w : (dh + 1) * w, :],
                        )
                    for w_sb, b_sb, dstT in (
                        (wk_sb, bk_sb, bt["kT"]),
                        (wq_sb, bq_sb, bt["qT"]),
                        (wv_sb, bv_sb, bt["vT"]),
                    ):
                        ps = psp.tile([P, QB], F32, tag="pp")
                        for dt_ in range(DT):
                            nc.tensor.matmul(
                                ps[:],
                                lhsT=(w_sb[:, dt_, :]),
                                rhs=(xh[:, dt_, :]),
                                start=(dt_ == 0),
                                stop=(dt_ == DT - 1),
                            )
                        nc.vector.tensor_scalar_add(
                            out=dstT[:, sblk * QB : (sblk + 1) * QB],
                            in0=ps[:],
                            scalar1=b_sb[:],
                        )
                        yield
                    # this sblk's slice of V transposed into v_st
                    pt = psp.tile([P, KPS, P], MDT, tag="pp")
                    for i in range(KPS):
                        kt = sblk * KPS + i
                        nc.tensor.transpose(
                            pt[:, i, :], vT[:, kt * P : (kt + 1) * P], ident[:]
                        )
                    tsrc = pt[:].rearrange("p n (h d) -> p n h d", h=2)
                    tdst = bass.AP(
                        tensor=v_st.tensor,
                        offset=v_st.offset + (sblk * KPS) * 130,
                        ap=[v_st.ap[0], [130, KPS], [65, 2], [1, 64]],
                    )
                    nc.vector.tensor_copy(out=tdst, in_=tsrc)
                    yield

            def drain(it, n=None):
                k = 0
                for _ in it:
                    k += 1
                    if n is not None and k >= n:
                        return True
                return False

            def wo_unit(b, st, ch, aoT):
                """One output-projection chunk (closure for deferred emission)."""
                def emit():
                    po = psp.tile([P, 512], F32, tag="pp", name="po")
                    nc.tensor.matmul(
                        po[:],
                        lhsT=(aoT[:, st * P : (st + 1) * P]),
                        rhs=(wo_sb[:, ch * 512 : (ch + 1) * 512]),
                        start=True,
                        stop=True,
                    )
                    o_sb = outp.tile([P, 512], F32, tag="o_sb", name="o_sb")
                    nc.vector.tensor_copy(out=o_sb[:], in_=po[:])
                    nc.sync.dma_start(
                        out_h.ap()[
                            b * S + st * P : b * S + (st + 1) * P,
                            ch * 512 : (ch + 1) * 512,
                        ],
                        o_sb[:],
                    )
                return emit

            def emit_body():
                from collections import deque

                woq = deque()
                epiq = deque()  # deferred q-block epilogues (top priority)

                def feed_bg(bg):
                    """Emit one background unit: pending epilogue first (it
                    releases the attn@V accumulator), then the projection
                    pipeline, else output-projection chunks. Returns the
                    (possibly exhausted) generator."""
                    if epiq:
                        epiq.popleft()()
                        return bg
                    if bg is not None:
                        if drain(bg, 1):
                            return bg
                        bg = None
                    n = 2 if len(woq) > 12 else 1
                    for _ in range(n):
                        if woq:
                            woq.popleft()()
                    return bg

                def chain_gens(*gens):
                    for g in gens:
                        if g is not None:
                            yield from g

                # ---- batch 0: emit only sblk0's units up front; the rest
                # interleaves into the attention group loop ----
                bt = alloc_batch_tiles()
                carry = proj_units(bt, 0)
                drain(carry, 4)  # K0, Q0, V0, T0

                for b in range(B):
                    # background: rest of this batch's projections, then the
                    # next batch's, interleaved into the group loop
                    bt_next = alloc_batch_tiles() if b + 1 < B else None
                    nxt = proj_units(bt_next, b + 1) if bt_next is not None else None
                    bg = chain_gens(carry, nxt)
                    carry = None
                    qT, kT, v_st = bt["qT"], bt["kT"], bt["v_st"]

                    aoT = aux.tile([P, S], MDT, tag="aoT")
                    for qb in range(NQB):
                        q0 = qb * QB
                        pav = psq.tile([P, 2 * QB], F32, tag="ps_av")
                        avq = deque()  # attn@V lags the exp pipeline 2 groups
                        for g in range(NG + 2):
                            if g < NG:
                                pss = psq.tile(
                                    [P, KG, 2, QB], F32, tag="ps_s",
                                    bufs=(2 if KG == 1 else 1),
                                )
                                for i in range(KG):
                                    kt = g * KG + i
                                    for h in range(2):
                                        nc.tensor.matmul(
                                            pss[:, i, h, :],
                                            lhsT=(
                                                kT[
                                                    h * 64 : (h + 1) * 64,
                                                    kt * P : (kt + 1) * P,
                                                ]
                                            ),
                                            rhs=(qT[h * 64 : (h + 1) * 64, q0 : q0 + QB]),
                                            start=True,
                                            stop=True,
                                            tile_position=(h * 64, 0),
                                        )
                                exps = expp.tile([P, KG, 2, QB], MDT, tag="exps")
                                nc.scalar.activation(
                                    out=exps[:].rearrange("p a b q -> p (a b q)"),
                                    in_=pss[:].rearrange("p a b q -> p (a b q)"),
                                    func=mybir.ActivationFunctionType.Exp,
                                    scale=SCALE,
                                )
                            if g < NG:
                                avq.append((g, exps))
                            # delayed attn@V, two groups behind
                            if g >= 2 and avq:
                                gp, exps_p = avq.popleft()
                                for i in range(KG):
                                    kt = gp * KG + i
                                    for h in range(2):
                                        nc.tensor.matmul(
                                            pav[0:65, h * QB : (h + 1) * QB],
                                            lhsT=(v_st[:, kt, h * 65 : (h + 1) * 65]),
                                            rhs=(exps_p[:, i, h, :]),
                                            start=(kt == 0),
                                            stop=(kt == KT - 1),
                                        )
                            # fill PE idle time under exp with background work
                            bg = feed_bg(bg)
                        def qb_epilogue(b=b, qb=qb, q0=q0, pav=pav, aoT=aoT):
                            # stage raw denominators at rows 0 (A), 32 (B)
                            nc.vector.tensor_copy(
                                out=sums[0:1, 0, :], in_=pav[64:65, 0:QB]
                            )
                            nc.vector.tensor_copy(
                                out=sums[32:33, 1, :], in_=pav[64:65, QB : 2 * QB]
                            )
                            # broadcast via all-ones matmul, reciprocal after
                            rec_sb = aux.tile([P, 2, QB], F32, tag="rec_sb")
                            for h in range(2):
                                prec = psp.tile([P, QB], F32, tag="pp", name="prec")
                                nc.tensor.matmul(
                                    prec[:],
                                    lhsT=(sel[:]),
                                    rhs=(sums[:, h, :]),
                                    start=True,
                                    stop=True,
                                )
                                nc.vector.reciprocal(out=rec_sb[:, h, :], in_=prec[:])
                            # normalize into aoT (head A rows 0:64, B 64:128)
                            nc.vector.tensor_tensor(
                                out=aoT[0:64, q0 : q0 + QB],
                                in0=pav[0:64, 0:QB],
                                in1=rec_sb[0:64, 0, :],
                                op=mybir.AluOpType.mult,
                            )
                            nc.vector.tensor_tensor(
                                out=aoT[64:128, q0 : q0 + QB],
                                in0=pav[0:64, QB : 2 * QB],
                                in1=rec_sb[64:128, 1, :],
                                op=mybir.AluOpType.mult,
                            )
                            # queue this q-block's output projection chunks
                            for st in range(qb * (QB // P), (qb + 1) * (QB // P)):
                                for ch in range(DOUT // 512):
                                    woq.append(wo_unit(b, st, ch, aoT))

                        # defer into the next q-block's group loop so the PE
                        # stream reaches the next scores without stalling
                        epiq.append(qb_epilogue)

                    # finish any remaining background projection work
                    while epiq:
                        epiq.popleft()()
                    if bg is not None:
                        drain(bg)
                        bg = None
                    bt = bt_next
                # tail: flush remaining output chunks
                while woq:
                    woq.popleft()()

            if loop_n is None:
                emit_body()
            else:
                with tc.For_i(0, loop_n, 1):
                    emit_body()

    nc.compile()
    return nc


def _prep_in_maps(inputs, n_cores=8):
    """Build per-core input dicts from the full problem inputs."""
    x = np.ascontiguousarray(np.asarray(inputs["inputs"], dtype=np.float32))
    Bb, Ss, Dd = x.shape
    xT = np.ascontiguousarray(x.reshape(Bb * Ss, Dd).T)  # [D, B*S]
    Wq = np.asarray(inputs["Wq"], dtype=np.float32)
    Wk = np.asarray(inputs["Wk"], dtype=np.float32)
    Wv = np.asarray(inputs["Wv"], dtype=np.float32)
    Wo = np.asarray(inputs["Wo"], dtype=np.float32)
    bq = np.asarray(inputs["bq"], dtype=np.float32)
    bk = np.asarray(inputs["bk"], dtype=np.float32)
    bv = np.asarray(inputs["bv"], dtype=np.float32)
    J = Wq.shape[1] // n_cores
    in_maps = []
    for c in range(n_cores):
        sl = slice(c * J, (c + 1) * J)
        in_maps.append(
            {
                "xt": xT,
                "wq": np.ascontiguousarray(Wq[:, sl]),
                "wk": np.ascontiguousarray(Wk[:, sl]),
                "wv": np.ascontiguousarray(Wv[:, sl]),
                "bq": np.ascontiguousarray(bq[sl]),
                "bk": np.ascontiguousarray(bk[sl]),
                "bv": np.ascontiguousarray(bv[sl]),
                "wo": np.ascontiguousarray(Wo[sl, :]),
            }
        )
    return in_maps


_NC_CACHE = {}


def kernel(**inputs) -> np.ndarray:
    from concourse.bass_utils import run_bass_kernel_spmd

    try:
        import jax

        jax.config.update("jax_compilation_cache_dir", "/tmp/jaxcache")
    except Exception:
        pass

    x = np.asarray(inputs["inputs"])
    Bb, Ss, Dd = x.shape
    DOUT = np.asarray(inputs["Wo"]).shape[1]

    key = (Bb, Ss, Dd, DOUT)
    if key not in _NC_CACHE:
        _NC_CACHE[key] = build_nc(S=Ss, D=Dd, DOUT=DOUT, B=Bb)
    nc = _NC_CACHE[key]

    in_maps = _prep_in_maps(inputs, n_cores=8)
    res = None
    for attempt in range(3):
        try:
            res = run_bass_kernel_spmd(nc, in_maps, core_ids=list(range(8)))
            break
        except Exception:
            # transient device wedges (NRT_EXEC_UNIT_UNRECOVERABLE) recover
            # on retry; re-raise only if persistent
            if attempt == 2:
                raise
            import time

            time.sleep(5)
    partial = np.stack([r["out"] for r in res.results], axis=0)
    out = partial.sum(axis=0, dtype=np.float64).astype(np.float32)
    out = out + np.asarray(inputs["bo"], dtype=np.float32)[None, :]
    return out.reshape(Bb, Ss, DOUT)

